# revision 84
# baseline (speedup 1.0000x reference)
"""Memory-efficient multi-head cross-attention on 8 TRN2 NeuronCores.

Sharding: batch (2) x head-block (4 heads each) across 8 cores, tensor-parallel
qkv projections.  Each core computes attention context for its 4 heads over
all 2048 query rows; one 8-wide AllToAll per 512-row chunk ships the
normalized context slices to the row-owning cores (cross-batch blocks are
masked out with a host-supplied 0/1 group mask so the program stays
SPMD-uniform), after which the full-depth o-projection, residual add and
LayerNorm for each core's own 128 rows per chunk are entirely local.

Schedule: the attention inner loop runs the 16.8M-exp/core stream on the
Scalar engine (~142us) against fp8 dual-row PE work of similar size;
everything else hides inside the stream:
 - per k-tile: two fp8 DoubleRow score matmuls (the k-pair slot dim reads
   the dense [128,S] operand twice through a stride-0 broadcast; the 2x
   double-count folds into the exp scale 0.0625), ONE wide exp [128,1024]
   covering both heads written as e5m2 (covers exp of scores to ~10.9, no
   max-subtraction), and per k-tile PAIR one DoubleRow context matmul per
   head (M=65: the ones-column accumulates the softmax denominator free).
 - projections / o-proj are fp8 DoubleRow with the model dim packed two
   128-blocks per partition (host-side layout), halving both PE time and
   input DMA bytes; K' q-block 0 and Q' chunk 0 are emitted first so the
   exp stream starts as early as the DMA allows.
 - the per-chunk 8-wide AllToAll ships fp8 context; cross-batch blocks
   are killed by an exact fp8 0/1 masked combine on the vector engine;
   the last chunk's hp0 AllToAll posts early with its load/combine
   popped late so no engine queue stalls behind the collective.
 - normalization (reciprocal of the two denominator rows at partitions
   0/32, one 33-row select matmul broadcast, fp8 multiply) is emitted in
   128-column pieces so no step head-blocks the PE queue; the last
   chunk's pieces ship on alternating DMA queues straight into the tail
   AllToAll.
 - residual+LN epilogue per chunk is DVE-only (bn_stats + Newton rsqrt).

kernel(**inputs) takes the FULL unsharded inputs and returns the FULL output.
"""

import sys
import types
from collections import deque

import ml_dtypes
import numpy as np

# ---------------------------------------------------------------------------
# Environment shims (must run before concourse imports are used)
# ---------------------------------------------------------------------------


def _install_ntff_shim():
    """Provide antenv.axon_hooks (absent in this image) so that
    run_bass_kernel_spmd(trace=True) can capture NTFF profiles via the
    axon ctypes hook. Harmless when tracing is off."""
    if "antenv.axon_hooks" in sys.modules:
        return
    hook = None
    try:
        from trn_agent_boot.trn_boot import _ntff_profile_via_ctypes

        hook = _ntff_profile_via_ctypes("/opt/axon/libaxon_pjrt.so")
    except Exception:
        hook = None
    mod = types.ModuleType("antenv.axon_hooks")
    mod.get_axon_ntff_profile_hook = lambda: hook
    mod.set_axon_ntff_profile_hook = lambda h: None
    sys.modules["antenv.axon_hooks"] = mod


_install_ntff_shim()

import concourse.bass as bass  # noqa: E402
import concourse.mybir as mybir  # noqa: E402
import concourse.tile as tile  # noqa: E402
from concourse.bass_utils import run_bass_kernel_spmd  # noqa: E402
from concourse.vector_clock import ScopedClock  # noqa: E402


def _patched_drain_and_barrier(self, tick_clock, wait_clock):
    """The walrus build in this image rejects a Drain carrying multiple sem
    waits ("Too many sync wait commands").  Emit the kernel-tail waits as
    standalone wait instructions on the sync engine instead, then drain."""
    nc = self.nc
    probe = nc.sync.nop(nofuse=True)
    wait_clock.add_sem_waits(probe.ins, ScopedClock({None: tick_clock.global_clock}))
    waits = list(probe.ins.sync_info.on_wait)
    probe.ins.sync_info.on_wait.clear()
    name2sem = {s.name: s for s in self.sems.allocated().values()}
    for w in waits:
        nc.sync.wait_ge(name2sem[w.ant_name], w.wait_value)
    nc.sync.drain()
    nc.all_engine_barrier()
    popped = nc._tile_sem_poison_stack.pop()
    assert popped is self._sem_poison
    nc.clear_and_free_semaphores(list(self.sems.allocated().values()))
    nc.all_engine_barrier()


tile.TileContext._drain_and_barrier = _patched_drain_and_barrier

# Max sem-waits this walrus build accepts on a single instruction.
_WAIT_LIMIT = 1


def _split_waits(nc, limit=_WAIT_LIMIT):
    """Hoist excess per-instruction sem waits into standalone EventSemaphore
    instructions (same engine, immediately preceding), since this walrus build
    rejects instructions carrying more than one sync wait."""
    n_split = 0
    for f in nc.m.functions:
        for bb in f.blocks:
            insts = bb.instructions
            i = 0
            while i < len(insts):
                inst = insts[i]
                si = getattr(inst, "sync_info", None)
                waits = si.on_wait if si is not None else None
                if waits is not None and len(waits) > limit:
                    excess = list(waits)[limit:]
                    del waits[limit:]
                    for w in excess:
                        ev = mybir.InstEventSemaphore(
                            name=f"I-{nc.next_id()}",
                            engine=inst.engine,
                            ins=[],
                            outs=[],
                        )
                        ev.sync_info = mybir.SyncInfo(on_wait=[w], on_update=[])
                        insts.insert(i, ev)
                        i += 1
                        n_split += 1
                i += 1
    return n_split


# ---------------------------------------------------------------------------
# Problem constants (hardcoded per the harness contract)
# ---------------------------------------------------------------------------
B = 2
SQ = 2048
SKV = 2048
D = 1024
NH = 16
DK = 64

NCORES = 8
GSZ = 4  # cores per batch group
HLOC = 4  # heads per core
DLOC = HLOC * DK  # 256 local context channels
P = 128
QCH = 512  # q chunk (matmul moving free dim)
NQC = SQ // QCH  # 4
NKT = SKV // P  # 16 k tiles
NMT = D // P  # 8 contraction tiles over model dim
QTR = 32  # rows per core per RS quarter

F32 = mybir.dt.float32
BF16 = mybir.dt.bfloat16
FP8 = mybir.dt.float8e4
FP8E5 = mybir.dt.float8e5
NPAIR = NKT // 2  # 8 k-tile pairs for DoubleRow ctx matmuls

LN_EPS = 1e-5

_CACHE = {}
LAST_RESULT = None


def _build(apply_gb):
    """Build the SPMD Bass program (identical on all 8 cores)."""
    nc = bass.Bass("TRN2", target_bir_lowering=False, num_devices=NCORES)

    # ---- kernel I/O (per-core shards supplied by the host) ----
    # fp8 activations/weights, model-dim packed in pairs (mt, dim1) for
    # DoubleRow projections: m = 128*(2i+j)+p at [.., i, p?, j, ..]
    xqT = nc.dram_tensor("xqT", [NQC * 4, P, 2, QCH], FP8, kind="ExternalInput")
    xkvT = nc.dram_tensor("xkvT", [4, P, 2, SKV], FP8, kind="ExternalInput")
    wqT = nc.dram_tensor("wqT", [P, 4, 2, DLOC], FP8, kind="ExternalInput")
    wkT = nc.dram_tensor("wkT", [P, 4, 2, DLOC], FP8, kind="ExternalInput")
    wvT = nc.dram_tensor("wvT", [P, 4, 2, DLOC], FP8, kind="ExternalInput")
    bqs = nc.dram_tensor("bqs", [DK, HLOC], F32, kind="ExternalInput")
    bks = nc.dram_tensor("bks", [DK, HLOC], F32, kind="ExternalInput")
    bvr = nc.dram_tensor("bvr", [1, DLOC], BF16, kind="ExternalInput")
    # o-proj weights per group-rank block r (0..3), head-pair t on the
    # DoubleRow slot dim
    woT = nc.dram_tensor("woT", [P, GSZ, 2, D], FP8, kind="ExternalInput")
    selc = nc.dram_tensor("selc", [DK, P], BF16, kind="ExternalInput")
    gmsk = nc.dram_tensor("gmsk", [P, 2], F32, kind="ExternalInput")
    # residual rows (query + b_o) for this core: [jc, 128, D]
    qres = nc.dram_tensor("qres", [P, NQC, D], BF16, kind="ExternalInput")
    if apply_gb:
        gam = nc.dram_tensor("gam", [P, D], F32, kind="ExternalInput")
        bet = nc.dram_tensor("bet", [P, D], F32, kind="ExternalInput")
    out = nc.dram_tensor("out", [NQC, P, D], BF16, kind="ExternalOutput")

    groups = [[0, 1, 2, 3], [4, 5, 6, 7]]
    Exp = mybir.ActivationFunctionType.Exp
    DROW = mybir.MatmulPerfMode.DoubleRow
    MUL = mybir.AluOpType.mult
    ADD = mybir.AluOpType.add
    SUB = mybir.AluOpType.subtract

    with tile.TileContext(nc) as tc:
        with (
            tc.tile_pool(name="cpool", bufs=1) as cpool,
            tc.tile_pool(name="spool", bufs=2) as spool,
            tc.tile_pool(name="dram", bufs=1, space="DRAM") as dram,
        ):
            # ---- persistent SBUF tensors ----
            wq_sb = cpool.tile([P, 4, 2, DLOC], FP8)
            wk_sb = cpool.tile([P, 4, 2, DLOC], FP8)
            wv_sb = cpool.tile([P, 4, 2, DLOC], FP8)
            bqs_sb = cpool.tile([DK, HLOC], F32)
            bks_sb = cpool.tile([DK, HLOC], F32)
            bvr_sb = cpool.tile([1, DLOC], BF16)
            onesP = cpool.tile([P, P], BF16)
            # row-select weights for the merged denominator broadcast:
            # col m of bcp = rdb[0] for m<64 else rdb[1]
            sel2 = cpool.tile([DK, P], BF16)
            warm = cpool.tile([1, 2], F32)
            warm2 = cpool.tile([1, 2], BF16)
            # Q'^T / K'^T in fp8 for DoubleRow scores: per head a dense
            # [128, S] slab whose lower 64 partitions hold the head's d
            # channels and whose upper 64 partitions are zero (dual-fp8
            # ldweights needs all 128 rows active; the score matmul reads
            # both operands through a stride-0 "k-pair" broadcast dim, which
            # doubles the score — folded into the exp scale).
            qt8 = cpool.tile([P, HLOC, SQ], FP8)
            kt8 = cpool.tile([P, HLOC, SKV], FP8)
            # V rows in fp8, paired k-tiles on the DoubleRow slot dim; per
            # head 64 V columns + a ones column at col 64 (softmax
            # denominator rides in psum partition 64 for free) + 15 pad
            # cols so the slot stride (80B) stays 16B-aligned.
            v2 = cpool.tile([P, NPAIR, HLOC, 2, 80], FP8)
            # normalized local context C^T: [d(128) x head-pair x q], fp8
            # (ships through the AllToAll and feeds the DoubleRow o-proj)
            ct_sb = cpool.tile([P, 2, SQ], FP8)
            wo_sb = cpool.tile([P, GSZ, 2, D], FP8)
            gm_sb = cpool.tile([P, 2], F32)
            # 0/1 group mask broadcast over the combine width, fp8 (exact)
            gmb8 = cpool.tile([P, 2, GSZ * 2 * P], FP8)
            qres_sb = cpool.tile([P, NQC, D], BF16)
            if apply_gb:
                gam_sb = cpool.tile([P, D], F32)
                bet_sb = cpool.tile([P, D], F32)
            xkv_p = [
                cpool.tile([P, 2, SKV], FP8, name=f"xkv_{i}") for i in range(4)
            ]
            # per-(chunk, m-pair) query slices so chunk 0's Q' only waits on
            # its own input slice
            xq_t = [
                [
                    cpool.tile([P, 2, QCH], FP8, name=f"xq_{j}_{i}")
                    for i in range(4)
                ]
                for j in range(NQC)
            ]
            # softmax-denominator scratch rows (memset so the merged
            # reciprocal over partitions 0:65 never sees uninitialized data)
            dcp = cpool.tile([P, QCH], F32)
            rdf = cpool.tile([P, QCH], F32)
            rdb = cpool.tile([P, QCH], BF16)

            # preload the exp table while DMAs stream
            nc.vector.memset(warm[:], 0.0)
            nc.scalar.activation(warm2[:], warm[:], Exp)
            # zero the dead upper halves of the fp8 score operands (dual-fp8
            # matmuls contract all 128 partitions); runs under the DMA-bound
            # prefix on otherwise-idle engines
            nc.gpsimd.memset(kt8[DK:P, :, :], 0.0)
            nc.vector.memset(qt8[DK:P, :, :], 0.0)
            # tiny warmup collective: absorbs inter-core launch skew during
            # the DMA-bound prefix instead of at the first real AllToAll
            wsync_in = dram.tile([NCORES, 16], F32, name="wsync_in")
            wsync_out = dram.tile([NCORES, 16], F32, name="wsync_out")
            wsrc = cpool.tile([NCORES, 16], F32)
            nc.vector.memset(wsrc[:], 0.0)
            nc.gpsimd.dma_start(wsync_in, wsrc[:])
            nc.gpsimd.collective_compute(
                "AllToAll",
                mybir.AluOpType.bypass,
                replica_groups=[list(range(NCORES))],
                ins=[wsync_in.opt()],
                outs=[wsync_out.opt()],
            )

            # ---- input DMAs: critical set (K', V, Q' ch0) first ----
            xkv_v = xkvT.ap()
            xq_v = xqT.ap()
            # strict two-queue priority: K'/V/Q'0 critical set first, the
            # rest queues behind it; gpsimd queue stays clear for collectives
            nc.sync.dma_start(wk_sb[:], wkT.ap())
            for i in range(2):
                nc.sync.dma_start(xkv_p[i][:], xkv_v[i])
            nc.scalar.dma_start(wv_sb[:], wvT.ap())
            nc.scalar.dma_start(wq_sb[:], wqT.ap())
            for i in range(2, 4):
                nc.scalar.dma_start(xkv_p[i][:], xkv_v[i])
            nc.scalar.dma_start(bks_sb[:], bks.ap())
            nc.scalar.dma_start(bqs_sb[:], bqs.ap())
            nc.scalar.dma_start(bvr_sb[:], bvr.ap())
            for i in range(4):
                nc.scalar.dma_start(xq_t[0][i][:], xq_v[i])
            # non-critical set: streams during chunk 0's attention
            for jc in range(1, NQC):
                for i in range(4):
                    q = nc.sync if jc == 1 else nc.scalar
                    q.dma_start(xq_t[jc][i][:], xq_v[4 * jc + i])
            nc.sync.dma_start(wo_sb[:], woT.ap())
            nc.scalar.dma_start(qres_sb[:], qres.ap())
            if apply_gb:
                nc.scalar.dma_start(gam_sb[:], gam.ap())
                nc.scalar.dma_start(bet_sb[:], bet.ap())
            nc.vector.memset(onesP[:], 1.0)
            nc.scalar.dma_start(sel2[:], selc.ap())
            nc.scalar.dma_start(gm_sb[:], gmsk.ap())
            # group-mask broadcast on the (otherwise idle) gpsimd engine
            nc.gpsimd.memset(gmb8[:], 1.0)
            for g in range(2):
                nc.gpsimd.tensor_scalar_mul(
                    gmb8[:, g, :], gmb8[:, g, :], gm_sb[:, g : g + 1]
                )
            nc.vector.memset(dcp[:], 1.0)
            nc.vector.memset(rdf[:], 1.0)
            nc.vector.memset(v2[:, :, :, :, 64:65], 1.0)

            # -------- Phase A: K' qc0, Q' ch0, K' qc1-3, V kt0-3 ----------
            # ordered so the first exp (needs K' qc0 + Q' ch0) fires as
            # early as possible; V kt4-15 stream as fillers inside chunk 0
            with tc.tile_pool(name="psA", bufs=8, space="PSUM") as psA:
                def k_block(qc):
                    tiles = []
                    for dt in range(2):
                        t = psA.tile(
                            [P, QCH], F32, tag="pj", name=f"pk_{qc}_{dt}"
                        )
                        tiles.append(t)
                    for i in range(4):
                        for dt in range(2):
                            nc.tensor.matmul(
                                tiles[dt][:],
                                lhsT=wk_sb[:, i, :, P * dt : P * dt + P],
                                rhs=xkv_p[i][:, :, QCH * qc : QCH * qc + QCH],
                                perf_mode=DROW,
                                start=(i == 0),
                                stop=(i == 3),
                            )
                    qsl = slice(QCH * qc, QCH * qc + QCH)
                    for dt in range(2):
                        for hh in range(2):
                            h = 2 * dt + hh
                            nc.vector.tensor_scalar(
                                kt8[0:DK, h, qsl],
                                tiles[dt][DK * hh : DK * hh + DK, :],
                                1.0,
                                bks_sb[:, h : h + 1],
                                MUL,
                                ADD,
                            )

                def v_tile_mms(ps, kt, lo, hi):
                    # lo/hi index m-PAIRS (0..4)
                    pv = ps[:, 0:DLOC]
                    for i in range(lo, hi):
                        nc.tensor.matmul(
                            pv,
                            lhsT=xkv_p[i][:, :, P * kt : P * kt + P],
                            rhs=wv_sb[:, i, :, :],
                            perf_mode=DROW,
                            start=(i == 0),
                            stop=False,
                        )
                    if hi == 4:
                        nc.tensor.matmul(
                            pv,
                            lhsT=onesP[0:1, :],
                            rhs=bvr_sb[0:1, :],
                            start=False,
                            stop=True,
                        )

                def v_copyback(ps, kt):
                    nc.vector.tensor_copy(
                        v2[:, kt // 2, :, kt % 2, 0:64],
                        ps[:, 0:DLOC].rearrange("p (h d) -> p h d", d=DK),
                    )

                k_block(0)
                pq = [
                    psA.tile([P, QCH], F32, tag="pj", name=f"pq_{i}")
                    for i in range(2)
                ]
                for i in range(4):
                    for dt in range(2):
                        nc.tensor.matmul(
                            pq[dt][:],
                            lhsT=wq_sb[:, i, :, P * dt : P * dt + P],
                            rhs=xq_t[0][i][:],
                            perf_mode=DROW,
                            start=(i == 0),
                            stop=(i == 3),
                        )
                for dt in range(2):
                    for hh in range(2):
                        h = 2 * dt + hh
                        nc.vector.tensor_scalar(
                            qt8[0:DK, h, 0:QCH],
                            pq[dt][DK * hh : DK * hh + DK, :],
                            1.0,
                            bqs_sb[:, h : h + 1],
                            MUL,
                            ADD,
                        )
                k_block(1)
                for kt in range(NKT // 2):
                    ps = psA.tile([P, QCH], F32, tag="pj", name=f"pv_{kt}")
                    v_tile_mms(ps, kt, 0, 4)
                    v_copyback(ps, kt)
                # re-align cores after the HBM-contended prefix so the
                # per-chunk AllToAlls don't inherit the DMA skew; the
                # (all-zero) result lands in a dead-but-read corner of
                # qt8 so chunk 1's first score matmul gates on it and
                # every core actually waits for the slowest
                wsync2_in = dram.tile([NCORES, 16], F32, name="wsync2_in")
                wsync2_out = dram.tile([NCORES, 16], F32, name="wsync2_out")
                nc.gpsimd.dma_start(wsync2_in, wsrc[:])
                nc.gpsimd.collective_compute(
                    "AllToAll",
                    mybir.AluOpType.bypass,
                    replica_groups=[list(range(NCORES))],
                    ins=[wsync2_in.opt()],
                    outs=[wsync2_out.opt()],
                )



            # ------- Phase B: exp-bound attention with fillers -------
            with (
                tc.tile_pool(name="opool", bufs=1) as opool,
                tc.tile_pool(name="psB", bufs=1, space="PSUM") as psB,
            ):
                fillers = deque()
                ctf_tiles = {}
                a2a_in_tiles = {}
                x_tiles = {}
                po_cache = {}
                aux_toggle = [0]

                def aux_tile(name):
                    # two auxiliary psum banks, round-robin, so back-to-back
                    # o-proj / broadcast matmuls double-buffer
                    aux_toggle[0] ^= 1
                    tag = "aux" if aux_toggle[0] else "vq"
                    return psB.tile([P, QCH], F32, tag=tag, bufs=1, name=name)

                def po_tile(jc, nch):
                    key = (jc, nch)
                    if key not in po_cache:
                        po_cache[key] = aux_tile(f"po_{jc}_{nch}")
                    return po_cache[key]

                # ---- filler generators ----
                def v_steps(kt):
                    ps_box = {}

                    def a():
                        ps_box["t"] = aux_tile(f"pvf_{kt}")
                        v_tile_mms(ps_box["t"], kt, 0, 2)

                    def b():
                        v_tile_mms(ps_box["t"], kt, 2, 4)

                    def c():
                        v_copyback(ps_box["t"], kt)

                    return [a, b, c]

                def qproj_steps(jc, dt):
                    qsl = slice(QCH * jc, QCH * jc + QCH)
                    ps_box = {}

                    def a(lo, hi):
                        if lo == 0:
                            ps_box["t"] = aux_tile(f"pqf_{jc}_{dt}")
                        for i in range(lo, hi):
                            nc.tensor.matmul(
                                ps_box["t"][:],
                                lhsT=wq_sb[:, i, :, P * dt : P * dt + P],
                                rhs=xq_t[jc][i][:],
                                perf_mode=DROW,
                                start=(i == 0),
                                stop=(i == 3),
                            )

                    def c():
                        for hh in range(2):
                            h = 2 * dt + hh
                            nc.vector.tensor_scalar(
                                qt8[0:DK, h, qsl],
                                ps_box["t"][DK * hh : DK * hh + DK, :],
                                1.0,
                                bqs_sb[:, h : h + 1],
                                MUL,
                                ADD,
                            )

                    return [lambda: a(0, 2), lambda: a(2, 4), c]

                def norm_head_emit(ctu, cx0, cx1):
                    """Free the ctx psum banks at a head-pair boundary:
                    copy denominator rows + unnormalized context to SBUF
                    (DVE only)."""
                    nc.vector.tensor_copy(dcp[0:1, :], cx0[64:65, :])
                    nc.vector.tensor_copy(dcp[32:33, :], cx1[64:65, :])
                    nc.vector.tensor_copy(ctu[0:64, :], cx0[0:64, :])
                    nc.vector.tensor_copy(ctu[64:128, :], cx1[0:64, :])

                def norm_steps(jc, hp, ctu, tail=False):
                    """Deferred tail of the normalization, in 128-column
                    pieces so no single step head-blocks the PE queue: per
                    piece a reciprocal of the two denominator rows
                    (partitions 0/32), a row-broadcast matmul, and the
                    normalize-multiply into fp8 ct."""
                    qoff = QCH * jc
                    box = {}

                    def alloc():
                        if tail:
                            # exp stream is done; borrow a score psum bank
                            # so the held po banks stay untouched
                            box["bcp"] = psB.tile(
                                [P, 2, QCH], F32, tag="s", bufs=2,
                                name=f"bcpt_{jc}_{hp}",
                            )[:, 0, :]
                        else:
                            box["bcp"] = aux_tile(f"bcp_{jc}_{hp}")

                    def rp(q4):
                        csl = slice(P * q4, P * q4 + P)
                        nc.vector.reciprocal(rdf[0:33, csl], dcp[0:33, csl])
                        nc.vector.tensor_copy(rdb[0:33, csl], rdf[0:33, csl])

                    def bp(q4):
                        if q4 == 0:
                            alloc()
                        csl = slice(P * q4, P * q4 + P)
                        nc.tensor.matmul(
                            box["bcp"][:, csl],
                            lhsT=sel2[0:33, :],
                            rhs=rdb[0:33, csl],
                        )
                        nc.vector.tensor_mul(
                            ct_sb[:, hp, qoff + P * q4 : qoff + P * q4 + P],
                            ctu[:, csl],
                            box["bcp"][:, csl],
                        )

                    return [
                        lambda q4=q4, f=f: f(q4)
                        for q4 in range(4)
                        for f in (rp, bp)
                    ]

                def ctf_tile(jc):
                    if jc not in ctf_tiles:
                        ctf_tiles[jc] = opool.tile(
                            [P, GSZ, 2, P], FP8, tag="cf", bufs=2,
                            name=f"ctf_{jc}",
                        )
                    return ctf_tiles[jc]

                def exchange_steps(jc, hps=(0, 1), skip_stage=False):
                    """Ship chunk jc's normalized context through one 8-wide
                    AllToAll (block j = our ctx for the q-rows owned by rank
                    j's position in its group), fp8 payload.  Cross-batch
                    blocks arrive as garbage; the masked combine (0/1 group
                    mask, exact in fp8) keeps the four same-group blocks on
                    the otherwise-idle gpsimd engine.  `hps` selects which
                    head-pair halves ship (the last chunk ships hp0 inside
                    the exp stream)."""
                    nh = len(hps)
                    sfx = f"{jc}_{hps[0]}{nh}"
                    a2a_in = dram.tile([NCORES, P, nh, P], FP8, name=f"a2a_in_{sfx}")
                    a2a_out = dram.tile([NCORES, P, nh, P], FP8, name=f"a2a_out_{sfx}")
                    ctf8 = opool.tile(
                        [P, NCORES, nh, P], FP8, tag=f"c8_{nh}", bufs=2
                    )
                    ctf = ctf_tile(jc)
                    h0 = hps[0]

                    def st(lo):
                        for j in range(lo, lo + 4):
                            qo = QCH * jc + P * (j % GSZ)
                            nc.sync.dma_start(
                                a2a_in[j],
                                ct_sb[:, h0 : h0 + nh, qo : qo + P],
                            )

                    def a2a():
                        nc.gpsimd.collective_compute(
                            "AllToAll",
                            mybir.AluOpType.bypass,
                            replica_groups=[list(range(NCORES))],
                            ins=[a2a_in.opt()],
                            outs=[a2a_out.opt()],
                        )

                    def load():
                        av = a2a_out.rearrange("r p t q -> p r t q")
                        nc.sync.dma_start(ctf8[:, 0:GSZ, :, :], av[:, 0:GSZ])
                        nc.sync.dma_start(ctf8[:, GSZ:, :, :], av[:, GSZ:])

                    fsz = GSZ * nh * P

                    def gmv(g):
                        return gmb8[:, g, 0:fsz].rearrange(
                            "p (a b c) -> p a b c", a=GSZ, b=nh
                        )

                    cdst = ctf[:, :, h0 : h0 + nh, :]

                    def comb1():
                        nc.vector.tensor_tensor(
                            ctf8[:, 0:GSZ, :, :], ctf8[:, 0:GSZ, :, :],
                            gmv(0), MUL,
                        )

                    def comb2():
                        nc.vector.tensor_tensor(
                            cdst, ctf8[:, GSZ : 2 * GSZ, :, :], gmv(1), MUL
                        )
                        nc.vector.tensor_add(cdst, cdst, ctf8[:, 0:GSZ, :, :])

                    a2a_in_tiles[(jc, hps[0], nh)] = a2a_in
                    if skip_stage:
                        return [a2a, load, comb1, comb2]
                    return [lambda: st(0), lambda: st(4), a2a, load, comb1, comb2]

                def oproj_steps(jc):
                    """Full-depth o-projection for our own 128 rows of chunk
                    jc: the four same-group peers' head-pair-paired context
                    slabs contract against fp8 o-weights in DoubleRow mode."""
                    ctf = ctf_tile(jc)

                    def grp(nch, lo):
                        po = po_tile(jc, nch)
                        nsl = slice(QCH * nch, QCH * nch + QCH)
                        for r in range(lo, lo + 2):
                            nc.tensor.matmul(
                                po[:],
                                lhsT=ctf[:, r, :, :],
                                rhs=wo_sb[:, r, :, nsl],
                                perf_mode=DROW,
                                start=(r == 0),
                                stop=(r == GSZ - 1),
                            )

                    return [
                        lambda: grp(0, 0),
                        lambda: grp(0, 2),
                        lambda: grp(1, 0),
                        lambda: grp(1, 2),
                    ]

                def oproj_adds(jc):
                    x_sb = opool.tile([P, D], F32, tag="x", bufs=2)
                    x_tiles[jc] = x_sb

                    def add(nch):
                        po = po_tile(jc, nch)
                        nsl = slice(QCH * nch, QCH * nch + QCH)
                        nc.vector.tensor_add(
                            x_sb[:, nsl], po[:], qres_sb[:, jc, nsl]
                        )

                    return [lambda nch=nch: add(nch) for nch in range(2)]

                def epilogue_steps(jc):
                    """LayerNorm for chunk jc's 128 rows.  DVE-only (rsqrt
                    via reciprocal seed + Newton) so the Scalar engine never
                    reloads activation tables."""
                    x_sb = x_tiles[jc]
                    y_sb = opool.tile([P, D], F32, tag="y", bufs=2)
                    yb_sb = opool.tile([P, D], BF16, tag="yb", bufs=2)
                    stat = spool.tile([P, 2, 6], F32, tag="stat")
                    mv = spool.tile([P, 2], F32, tag="mv")
                    var = spool.tile([P, 1], F32, tag="var")
                    yy = spool.tile([P, 1], F32, tag="yy")
                    tt = spool.tile([P, 1], F32, tag="tt")
                    vh = spool.tile([P, 1], F32, tag="vh")
                    mu = mv[:, 0:1]

                    def e2():
                        # mean/variance via the BN stats unit (512-wide max)
                        nc.vector.bn_stats(stat[:, 0, :], x_sb[:, 0 : D // 2])
                        nc.vector.bn_stats(stat[:, 1, :], x_sb[:, D // 2 :])

                    def e3():
                        nc.vector.bn_aggr(mv[:], stat[:])

                    def e4():
                        nc.vector.tensor_scalar_add(var[:], mv[:, 1:2], LN_EPS)
                        nc.vector.reciprocal(yy[:], var[:])
                        nc.vector.tensor_scalar_mul(vh[:], var[:], -0.5)
                        for _ in range(3):
                            nc.vector.tensor_mul(tt[:], yy[:], yy[:])
                            nc.vector.tensor_scalar(tt[:], tt[:], vh[:], 1.5, MUL, ADD)
                            nc.vector.tensor_mul(yy[:], yy[:], tt[:])

                    def e5():
                        if apply_gb:
                            nc.vector.tensor_scalar(
                                y_sb[:], x_sb[:], mu, yy[:], SUB, MUL
                            )
                            nc.vector.tensor_mul(y_sb[:], y_sb[:], gam_sb[:])
                            nc.vector.tensor_add(yb_sb[:], y_sb[:], bet_sb[:])
                            nc.sync.dma_start(out.ap()[jc], yb_sb[:])
                        else:
                            for h in range(2):
                                csl = slice(D // 2 * h, D // 2 * (h + 1))
                                nc.vector.tensor_scalar(
                                    yb_sb[:, csl], x_sb[:, csl], mu, yy[:], SUB, MUL
                                )
                                nc.sync.dma_start(out.ap()[jc][:, csl], yb_sb[:, csl])

                    return [e2, e3, e4, e5]

                # K' q-blocks 2-3 deferred from the prefix (not needed
                # until kt8/kt12); both aux banks held mm(0)->cb, released
                # before any v/q filler allocates
                def k_steps(qc):
                    boxes = {}

                    def mm(dt):
                        t = aux_tile(f"pkf_{qc}_{dt}")
                        boxes[dt] = t
                        for i in range(4):
                            nc.tensor.matmul(
                                t[:],
                                lhsT=wk_sb[:, i, :, P * dt : P * dt + P],
                                rhs=xkv_p[i][:, :, QCH * qc : QCH * qc + QCH],
                                perf_mode=DROW,
                                start=(i == 0),
                                stop=(i == 3),
                            )

                    def cb():
                        qsl = slice(QCH * qc, QCH * qc + QCH)
                        for dt in range(2):
                            for hh in range(2):
                                h = 2 * dt + hh
                                nc.vector.tensor_scalar(
                                    kt8[0:DK, h, qsl],
                                    boxes[dt][DK * hh : DK * hh + DK, :],
                                    1.0,
                                    bks_sb[:, h : h + 1],
                                    MUL,
                                    ADD,
                                )

                    return [lambda: mm(0), lambda: mm(1), cb]

                # K' 2-3, V k-tiles 8-15, Q' chunks 1-3 fill chunk 0's
                # attention (popped 3/kt there so every emission lands
                # before its consumer)
                fillers.extend(k_steps(2))
                fillers.extend(k_steps(3))
                for kt in range(NKT // 2, NKT):
                    fillers.extend(v_steps(kt))
                for dt in range(2):
                    fillers.extend(qproj_steps(1, dt))

                # ---- the exp-bound attention loop ----
                def post_steps(k):
                    return oproj_steps(k) + oproj_adds(k) + epilogue_steps(k)

                pend = {}
                # cross-stream carry: the last ctx pair + norm_head copies
                # of stream N are emitted after stream N+1's first scores
                # (PE never head-blocks on exp(kt15) at a boundary); the
                # new stream's cx psum tiles are allocated AFTER the carry
                # flush so the pool orders bank reuse behind the old reads
                xcarry = [None]
                for jc in range(NQC):
                    if jc >= 1:
                        # norm steps must stay contiguous: the bcp psum
                        # bank allocated at the first bp piece shares the
                        # two aux banks with post_steps' held po tiles
                        nst = norm_steps(jc - 1, 1, pend[(jc - 1, 1)])
                        fillers.extend(nst)
                        if jc == NQC - 1:
                            # chunk 1's post-exchange work: 1.5 chunks after
                            # its AllToAll so collective jitter never
                            # head-blocks the PE queue
                            fillers.extend(post_steps(1))
                        fillers.extend(exchange_steps(jc - 1))
                        if jc + 1 < NQC:
                            # Q' for chunk jc+1: chunk 0 is PE-bound, later
                            # chunks have spare PE slack
                            for dt in range(2):
                                fillers.extend(qproj_steps(jc + 1, dt))
                    qsl = slice(QCH * jc, QCH * jc + QCH)
                    for hp in range(2):
                        if hp == 1:
                            nst = norm_steps(jc, 0, pend[(jc, 0)])
                            fillers.extend(nst)
                            if jc == NQC - 2:
                                fillers.extend(post_steps(0))
                            if jc == NQC - 1:
                                # ship hp0 of the last chunk as early as
                                # possible (its AllToAll must complete
                                # before the tail's o-proj), but keep the
                                # load/combine late so the DVE queue never
                                # stalls behind the collective
                                ex0 = exchange_steps(jc, hps=(0,))
                                fillers.extend(ex0[:3])
                                fillers.extend(post_steps(jc - 1))
                                fillers.extend(ex0[3:])
                        cxb = {}
                        h0, h1 = 2 * hp, 2 * hp + 1
                        p2 = None
                        pend_ctx = None

                        def mk_ctx(pr, p2c, st, sp, cxb=cxb, h0=h0, h1=h1):
                            def go():
                                for i, cx, h in (
                                    (0, cxb["c0"], h0),
                                    (1, cxb["c1"], h1),
                                ):
                                    nc.tensor.matmul(
                                        cx[0:65, :],
                                        lhsT=v2[:, pr, h, :, 0:65],
                                        rhs=p2c[:, i, :, :],
                                        perf_mode=DROW,
                                        start=st,
                                        stop=sp,
                                    )

                            return go

                        for kt in range(NKT):
                            ksl = slice(P * kt, P * kt + P)
                            s = psB.tile([P, 2, QCH], F32, tag="s", bufs=2)
                            for i, h in ((0, h0), (1, h1)):
                                nc.tensor.matmul(
                                    s[:, i, :],
                                    lhsT=kt8[:, h, ksl]
                                    .unsqueeze(1)
                                    .broadcast_to([P, 2, P]),
                                    rhs=qt8[:, h, qsl]
                                    .unsqueeze(1)
                                    .broadcast_to([P, 2, QCH]),
                                    perf_mode=DROW,
                                )
                            # the ctx pair for the PREVIOUS k-tile pair is
                            # emitted after this k-tile's scores: its exp
                            # dependency resolves while the scores stream,
                            # so the in-order PE queue never head-blocks
                            if kt == 0:
                                if xcarry[0] is not None:
                                    xcarry[0]()
                                    xcarry[0] = None
                                cxb["c0"] = psB.tile(
                                    [P, QCH], F32, tag="ctx0", bufs=1,
                                    name=f"cx0_{jc}_{hp}",
                                )
                                cxb["c1"] = psB.tile(
                                    [P, QCH], F32, tag="ctx1", bufs=1,
                                    name=f"cx1_{jc}_{hp}",
                                )
                            if pend_ctx is not None:
                                pend_ctx()
                                pend_ctx = None
                            if kt % 2 == 0:
                                p2 = spool.tile(
                                    [P, 2, 2, QCH], FP8E5, tag="p", bufs=6
                                )
                            # scale: 1/sqrt(dk)=0.125 times 0.5 for the
                            # stride-0 double-count of the score contraction.
                            # e5m2 output: covers exp of scores up to 10.9
                            # with no max-subtraction needed.
                            nc.scalar.activation(
                                p2[:, :, kt % 2, :], s[:], Exp, scale=0.0625
                            )
                            if kt % 2 == 1:
                                pend_ctx = mk_ctx(
                                    kt // 2, p2, kt == 1, kt == NKT - 1
                                )
                            # keep the first/last k-tiles filler-free so the
                            # exp stream never competes at boundaries
                            if 2 <= kt <= 14:
                                if jc == 0 and hp == 0 and len(fillers) > 4:
                                    n_pop = 3
                                else:
                                    n_pop = 2 if len(fillers) > 4 else 1
                                for _ in range(n_pop):
                                    if fillers:
                                        fillers.popleft()()
                        ctu = spool.tile(
                            [P, QCH], BF16, tag="ctu", bufs=3,
                            name=f"ctu_{jc}_{hp}",
                        )
                        pend[(jc, hp)] = ctu

                        def mk_carry(pc=pend_ctx, ct=ctu,
                                     c0=cxb["c0"], c1=cxb["c1"]):
                            def go():
                                pc()
                                norm_head_emit(ct, c0, c1)

                            return go

                        xcarry[0] = mk_carry()
                        pend_ctx = None

                # ---- tail ----
                if xcarry[0] is not None:
                    xcarry[0]()
                    xcarry[0] = None
                while fillers:
                    fillers.popleft()()
                jl = NQC - 1
                norm_tail = norm_steps(jl, 1, pend[(jl, 1)], tail=True)
                # per 128-column piece: recip -> broadcast+normalize -> ship
                # (alternating DMA queues so the stage DMAs overlap)
                ex = exchange_steps(jl, hps=(1,), skip_stage=True)
                a2a_in_t = a2a_in_tiles[(jl, 1, 1)]
                qoff = QCH * jl
                for q4 in range(GSZ):
                    norm_tail[2 * q4]()
                    norm_tail[2 * q4 + 1]()
                    qq = nc.sync if q4 % 2 == 0 else nc.scalar
                    for j in (q4, q4 + GSZ):
                        qq.dma_start(
                            a2a_in_t[j],
                            ct_sb[:, 1:2, qoff + P * q4 : qoff + P * q4 + P],
                        )
                for step in (
                    ex + oproj_steps(jl) + oproj_adds(jl) + epilogue_steps(jl)
                ):
                    step()

    _split_waits(nc)
    return nc


def _prep_inputs(query, key_value, W_qkv, b_qkv, W_o, b_o, ln_gamma, ln_beta,
                 apply_gb):
    bf16 = ml_dtypes.bfloat16
    f32 = np.float32
    query = np.asarray(query, f32)
    key_value = np.asarray(key_value, f32)
    W_qkv = np.asarray(W_qkv, f32)
    b_qkv = np.asarray(b_qkv, f32)
    W_o = np.asarray(W_o, f32)
    b_o = np.asarray(b_o, f32)
    ln_gamma = np.asarray(ln_gamma, f32)
    ln_beta = np.asarray(ln_beta, f32)

    fp8 = ml_dtypes.float8_e4m3
    Wq, Wk, Wv = W_qkv[:D], W_qkv[D : 2 * D], W_qkv[2 * D :]
    bq, bk, bv = b_qkv[:D], b_qkv[D : 2 * D], b_qkv[2 * D :]

    gam = np.ascontiguousarray(np.broadcast_to(ln_gamma, (P, D))).astype(f32)
    bet = np.ascontiguousarray(np.broadcast_to(ln_beta, (P, D))).astype(f32)
    sel_const = np.zeros((DK, P), f32)
    sel_const[0, 0:DK] = 1.0
    sel_const[32, DK:P] = 1.0
    sel_const = sel_const.astype(bf16)

    def pack_w(W):
        # [1024, DLOC] -> [P, 4, 2, DLOC] fp8, m = 128*(2i+j)+p
        return np.ascontiguousarray(
            W.T.reshape(4, 2, P, DLOC).transpose(2, 0, 1, 3)
        ).astype(fp8)

    # o-proj weights per group-rank block r, head-pair t on the slot dim
    wo2 = np.ascontiguousarray(
        W_o.T.reshape(GSZ, 2, P, D).transpose(2, 0, 1, 3)
    ).astype(fp8)

    xqT = [
        np.ascontiguousarray(
            query[b].T.reshape(4, 2, P, NQC, QCH)
            .transpose(3, 0, 2, 1, 4)
            .reshape(NQC * 4, P, 2, QCH)
        ).astype(fp8)
        for b in range(B)
    ]
    xkvT = [
        np.ascontiguousarray(
            key_value[b].T.reshape(4, 2, P, SKV).transpose(0, 2, 1, 3)
        ).astype(fp8)
        for b in range(B)
    ]

    in_maps = []
    for c in range(NCORES):
        b = c // GSZ
        hb = c % GSZ
        jb = c % GSZ
        sl = slice(DLOC * hb, DLOC * hb + DLOC)
        gm = np.zeros((P, 2), f32)
        gm[:, b] = 1.0
        # this core owns rows 512*jc + 128*jb .. +128 of each chunk jc
        res_rows = np.stack(
            [
                query[b, QCH * jc + P * jb : QCH * jc + P * jb + P]
                + b_o[None, :]
                for jc in range(NQC)
            ]
        ).transpose(1, 0, 2)
        im = {
            "xqT": xqT[b],
            "xkvT": xkvT[b],
            "wqT": pack_w(Wq[sl]),
            "wkT": pack_w(Wk[sl]),
            "wvT": pack_w(Wv[sl]),
            "bqs": np.ascontiguousarray(bq[sl].reshape(HLOC, DK).T).astype(f32),
            "bks": np.ascontiguousarray(bk[sl].reshape(HLOC, DK).T).astype(f32),
            "bvr": bv[sl][None, :].astype(bf16),
            "woT": wo2,
            "selc": sel_const,
            "gmsk": gm,
            "qres": res_rows.astype(bf16),
        }
        if apply_gb:
            im["gam"] = gam
            im["bet"] = bet
        in_maps.append(im)
    return in_maps


def kernel(query, key_value, W_qkv, b_qkv, W_o, b_o, ln_gamma, ln_beta):
    global LAST_RESULT
    apply_gb = not (
        np.all(np.asarray(ln_gamma) == 1.0) and np.all(np.asarray(ln_beta) == 0.0)
    )
    key = ("nc", apply_gb)
    if key not in _CACHE:
        _CACHE[key] = _build(apply_gb)
    nc = _CACHE[key]
    in_maps = _prep_inputs(
        query, key_value, W_qkv, b_qkv, W_o, b_o, ln_gamma, ln_beta, apply_gb
    )
    res = run_bass_kernel_spmd(nc, in_maps, core_ids=list(range(NCORES)))
    LAST_RESULT = res
    full = np.empty((B, SQ, D), np.float32)
    for c in range(NCORES):
        b = c // GSZ
        jb = c % GSZ
        o = np.asarray(res.results[c]["out"], np.float32)  # [NQC, P, D]
        for jc in range(NQC):
            r0 = QCH * jc + P * jb
            full[b, r0 : r0 + P] = o[jc]
    return full



# revision 85
# speedup vs baseline: 1.0295x; 1.0295x over previous
"""Memory-efficient multi-head cross-attention on 8 TRN2 NeuronCores.

Sharding: batch (2) x head-block (4 heads each) across 8 cores, tensor-parallel
qkv projections.  Each core computes attention context for its 4 heads over
all 2048 query rows; one 8-wide AllToAll per 512-row chunk ships the
normalized context slices to the row-owning cores (cross-batch blocks are
masked out with a host-supplied 0/1 group mask so the program stays
SPMD-uniform), after which the full-depth o-projection, residual add and
LayerNorm for each core's own 128 rows per chunk are entirely local.

Schedule: the attention inner loop runs the 16.8M-exp/core stream on the
Scalar engine (~142us) against fp8 dual-row PE work of similar size;
everything else hides inside the stream:
 - per k-tile: two fp8 DoubleRow score matmuls (the k-pair slot dim reads
   the dense [128,S] operand twice through a stride-0 broadcast; the 2x
   double-count folds into the exp scale 0.0625), ONE wide exp [128,1024]
   covering both heads written as e5m2 (covers exp of scores to ~10.9, no
   max-subtraction), and per k-tile PAIR one DoubleRow context matmul per
   head (M=65: the ones-column accumulates the softmax denominator free).
 - projections / o-proj are fp8 DoubleRow with the model dim packed two
   128-blocks per partition (host-side layout), halving both PE time and
   input DMA bytes; K' q-block 0 and Q' chunk 0 are emitted first so the
   exp stream starts as early as the DMA allows.
 - the per-chunk 8-wide AllToAll ships fp8 context; cross-batch blocks
   are killed by an exact fp8 0/1 masked combine on the vector engine;
   the last chunk's hp0 AllToAll posts early with its load/combine
   popped late so no engine queue stalls behind the collective.
 - normalization (reciprocal of the two denominator rows at partitions
   0/32, one 33-row select matmul broadcast, fp8 multiply) is emitted in
   128-column pieces so no step head-blocks the PE queue; the last
   chunk's pieces ship on alternating DMA queues straight into the tail
   AllToAll.
 - residual+LN epilogue per chunk is DVE-only (bn_stats + Newton rsqrt).

kernel(**inputs) takes the FULL unsharded inputs and returns the FULL output.
"""

import sys
import types
from collections import deque

import ml_dtypes
import numpy as np

# ---------------------------------------------------------------------------
# Environment shims (must run before concourse imports are used)
# ---------------------------------------------------------------------------


def _install_ntff_shim():
    """Provide antenv.axon_hooks (absent in this image) so that
    run_bass_kernel_spmd(trace=True) can capture NTFF profiles via the
    axon ctypes hook. Harmless when tracing is off."""
    if "antenv.axon_hooks" in sys.modules:
        return
    hook = None
    try:
        from trn_agent_boot.trn_boot import _ntff_profile_via_ctypes

        hook = _ntff_profile_via_ctypes("/opt/axon/libaxon_pjrt.so")
    except Exception:
        hook = None
    mod = types.ModuleType("antenv.axon_hooks")
    mod.get_axon_ntff_profile_hook = lambda: hook
    mod.set_axon_ntff_profile_hook = lambda h: None
    sys.modules["antenv.axon_hooks"] = mod


_install_ntff_shim()

import concourse.bass as bass  # noqa: E402
import concourse.mybir as mybir  # noqa: E402
import concourse.tile as tile  # noqa: E402
from concourse.bass_utils import run_bass_kernel_spmd  # noqa: E402
from concourse.vector_clock import ScopedClock  # noqa: E402


def _patched_drain_and_barrier(self, tick_clock, wait_clock):
    """The walrus build in this image rejects a Drain carrying multiple sem
    waits ("Too many sync wait commands").  Emit the kernel-tail waits as
    standalone wait instructions on the sync engine instead, then drain."""
    nc = self.nc
    probe = nc.sync.nop(nofuse=True)
    wait_clock.add_sem_waits(probe.ins, ScopedClock({None: tick_clock.global_clock}))
    waits = list(probe.ins.sync_info.on_wait)
    probe.ins.sync_info.on_wait.clear()
    name2sem = {s.name: s for s in self.sems.allocated().values()}
    for w in waits:
        nc.sync.wait_ge(name2sem[w.ant_name], w.wait_value)
    nc.sync.drain()
    nc.all_engine_barrier()
    popped = nc._tile_sem_poison_stack.pop()
    assert popped is self._sem_poison
    nc.clear_and_free_semaphores(list(self.sems.allocated().values()))
    nc.all_engine_barrier()


tile.TileContext._drain_and_barrier = _patched_drain_and_barrier

# Max sem-waits this walrus build accepts on a single instruction.
_WAIT_LIMIT = 1


def _split_waits(nc, limit=_WAIT_LIMIT):
    """Hoist excess per-instruction sem waits into standalone EventSemaphore
    instructions (same engine, immediately preceding), since this walrus build
    rejects instructions carrying more than one sync wait."""
    n_split = 0
    for f in nc.m.functions:
        for bb in f.blocks:
            insts = bb.instructions
            i = 0
            while i < len(insts):
                inst = insts[i]
                si = getattr(inst, "sync_info", None)
                waits = si.on_wait if si is not None else None
                if waits is not None and len(waits) > limit:
                    excess = list(waits)[limit:]
                    del waits[limit:]
                    for w in excess:
                        ev = mybir.InstEventSemaphore(
                            name=f"I-{nc.next_id()}",
                            engine=inst.engine,
                            ins=[],
                            outs=[],
                        )
                        ev.sync_info = mybir.SyncInfo(on_wait=[w], on_update=[])
                        insts.insert(i, ev)
                        i += 1
                        n_split += 1
                i += 1
    return n_split


# ---------------------------------------------------------------------------
# Problem constants (hardcoded per the harness contract)
# ---------------------------------------------------------------------------
B = 2
SQ = 2048
SKV = 2048
D = 1024
NH = 16
DK = 64

NCORES = 8
GSZ = 4  # cores per batch group
HLOC = 4  # heads per core
DLOC = HLOC * DK  # 256 local context channels
P = 128
QCH = 512  # q chunk (matmul moving free dim)
NQC = SQ // QCH  # 4
NKT = SKV // P  # 16 k tiles
NMT = D // P  # 8 contraction tiles over model dim
QTR = 32  # rows per core per RS quarter

F32 = mybir.dt.float32
BF16 = mybir.dt.bfloat16
FP8 = mybir.dt.float8e4
FP8E5 = mybir.dt.float8e5
NPAIR = NKT // 2  # 8 k-tile pairs for DoubleRow ctx matmuls

LN_EPS = 1e-5

_CACHE = {}
LAST_RESULT = None


def _build(apply_gb):
    """Build the SPMD Bass program (identical on all 8 cores)."""
    nc = bass.Bass("TRN2", target_bir_lowering=False, num_devices=NCORES)

    # ---- kernel I/O (per-core shards supplied by the host) ----
    # fp8 activations/weights, model-dim packed in pairs (mt, dim1) for
    # DoubleRow projections: m = 128*(2i+j)+p at [.., i, p?, j, ..]
    xqT = nc.dram_tensor("xqT", [NQC * 4, P, 2, QCH], FP8, kind="ExternalInput")
    xkvT = nc.dram_tensor("xkvT", [4, P, 2, SKV], FP8, kind="ExternalInput")
    wqT = nc.dram_tensor("wqT", [P, 4, 2, DLOC], FP8, kind="ExternalInput")
    wkT = nc.dram_tensor("wkT", [P, 4, 2, DLOC], FP8, kind="ExternalInput")
    wvT = nc.dram_tensor("wvT", [P, 4, 2, DLOC], FP8, kind="ExternalInput")
    bqs = nc.dram_tensor("bqs", [DK, HLOC], F32, kind="ExternalInput")
    bks = nc.dram_tensor("bks", [DK, HLOC], F32, kind="ExternalInput")
    bvr = nc.dram_tensor("bvr", [1, DLOC], BF16, kind="ExternalInput")
    # o-proj weights per group-rank block r (0..3), head-pair t on the
    # DoubleRow slot dim
    woT = nc.dram_tensor("woT", [P, GSZ, 2, D], FP8, kind="ExternalInput")
    selc = nc.dram_tensor("selc", [DK, P], BF16, kind="ExternalInput")
    gmsk = nc.dram_tensor("gmsk", [P, 2], F32, kind="ExternalInput")
    # residual rows (query + b_o) for this core: [jc, 128, D]
    qres = nc.dram_tensor("qres", [P, NQC, D], BF16, kind="ExternalInput")
    if apply_gb:
        gam = nc.dram_tensor("gam", [P, D], F32, kind="ExternalInput")
        bet = nc.dram_tensor("bet", [P, D], F32, kind="ExternalInput")
    out = nc.dram_tensor("out", [NQC, P, D], BF16, kind="ExternalOutput")

    groups = [[0, 1, 2, 3], [4, 5, 6, 7]]
    Exp = mybir.ActivationFunctionType.Exp
    DROW = mybir.MatmulPerfMode.DoubleRow
    MUL = mybir.AluOpType.mult
    ADD = mybir.AluOpType.add
    SUB = mybir.AluOpType.subtract

    with tile.TileContext(nc) as tc:
        with (
            tc.tile_pool(name="cpool", bufs=1) as cpool,
            tc.tile_pool(name="spool", bufs=2) as spool,
            tc.tile_pool(name="dram", bufs=1, space="DRAM") as dram,
        ):
            # ---- persistent SBUF tensors ----
            wq_sb = cpool.tile([P, 4, 2, DLOC], FP8)
            wk_sb = cpool.tile([P, 4, 2, DLOC], FP8)
            wv_sb = cpool.tile([P, 4, 2, DLOC], FP8)
            bqs_sb = cpool.tile([DK, HLOC], F32)
            bks_sb = cpool.tile([DK, HLOC], F32)
            bvr_sb = cpool.tile([1, DLOC], BF16)
            onesP = cpool.tile([P, P], BF16)
            # row-select weights for the merged denominator broadcast:
            # col m of bcp = rdb[0] for m<64 else rdb[1]
            sel2 = cpool.tile([DK, P], BF16)
            warm = cpool.tile([1, 2], F32)
            warm2 = cpool.tile([1, 2], BF16)
            # Q'^T / K'^T in fp8 for DoubleRow scores: per head a dense
            # [128, S] slab whose lower 64 partitions hold the head's d
            # channels and whose upper 64 partitions are zero (dual-fp8
            # ldweights needs all 128 rows active; the score matmul reads
            # both operands through a stride-0 "k-pair" broadcast dim, which
            # doubles the score — folded into the exp scale).
            qt8 = cpool.tile([P, HLOC, SQ], FP8)
            kt8 = cpool.tile([P, HLOC, SKV], FP8)
            # V rows in fp8, paired k-tiles on the DoubleRow slot dim; per
            # head 64 V columns + a ones column at col 64 (softmax
            # denominator rides in psum partition 64 for free) + 15 pad
            # cols so the slot stride (80B) stays 16B-aligned.
            v2 = cpool.tile([P, NPAIR, HLOC, 2, 80], FP8)
            # normalized local context C^T: [d(128) x head-pair x q], fp8
            # (ships through the AllToAll and feeds the DoubleRow o-proj)
            ct_sb = cpool.tile([P, 2, SQ], FP8)
            wo_sb = cpool.tile([P, GSZ, 2, D], FP8)
            gm_sb = cpool.tile([P, 2], F32)
            # 0/1 group mask broadcast over the combine width, fp8 (exact)
            gmb8 = cpool.tile([P, 2, GSZ * 2 * P], FP8)
            qres_sb = cpool.tile([P, NQC, D], BF16)
            if apply_gb:
                gam_sb = cpool.tile([P, D], F32)
                bet_sb = cpool.tile([P, D], F32)
            xkv_p = [
                cpool.tile([P, 2, SKV], FP8, name=f"xkv_{i}") for i in range(4)
            ]
            # per-(chunk, m-pair) query slices so chunk 0's Q' only waits on
            # its own input slice
            xq_t = [
                [
                    cpool.tile([P, 2, QCH], FP8, name=f"xq_{j}_{i}")
                    for i in range(4)
                ]
                for j in range(NQC)
            ]
            # softmax-denominator scratch rows (memset so the merged
            # reciprocal over partitions 0:65 never sees uninitialized data)
            dcp = cpool.tile([P, QCH], F32)
            rdf = cpool.tile([P, QCH], F32)
            rdb = cpool.tile([P, QCH], BF16)

            # preload the exp table while DMAs stream
            nc.vector.memset(warm[:], 0.0)
            nc.scalar.activation(warm2[:], warm[:], Exp)
            # zero the dead upper halves of the fp8 score operands (dual-fp8
            # matmuls contract all 128 partitions); runs under the DMA-bound
            # prefix on otherwise-idle engines
            nc.gpsimd.memset(kt8[DK:P, :, :], 0.0)
            nc.vector.memset(qt8[DK:P, :, :], 0.0)
            # tiny warmup collective: absorbs inter-core launch skew during
            # the DMA-bound prefix instead of at the first real AllToAll
            wsync_in = dram.tile([NCORES, 16], F32, name="wsync_in")
            wsync_out = dram.tile([NCORES, 16], F32, name="wsync_out")
            wsrc = cpool.tile([NCORES, 16], F32)
            nc.vector.memset(wsrc[:], 0.0)
            nc.gpsimd.dma_start(wsync_in, wsrc[:])
            nc.gpsimd.collective_compute(
                "AllToAll",
                mybir.AluOpType.bypass,
                replica_groups=[list(range(NCORES))],
                ins=[wsync_in.opt()],
                outs=[wsync_out.opt()],
            )

            # ---- input DMAs: critical set (K', V, Q' ch0) first ----
            xkv_v = xkvT.ap()
            xq_v = xqT.ap()
            # strict two-queue priority: K'/V/Q'0 critical set first, the
            # rest queues behind it; gpsimd queue stays clear for collectives
            nc.sync.dma_start(wk_sb[:], wkT.ap())
            for i in range(2):
                nc.sync.dma_start(xkv_p[i][:], xkv_v[i])
            nc.scalar.dma_start(wv_sb[:], wvT.ap())
            nc.scalar.dma_start(wq_sb[:], wqT.ap())
            for i in range(2, 4):
                nc.scalar.dma_start(xkv_p[i][:], xkv_v[i])
            nc.scalar.dma_start(bks_sb[:], bks.ap())
            nc.scalar.dma_start(bqs_sb[:], bqs.ap())
            nc.scalar.dma_start(bvr_sb[:], bvr.ap())
            for i in range(4):
                nc.scalar.dma_start(xq_t[0][i][:], xq_v[i])
            # non-critical set: streams during chunk 0's attention
            for jc in range(1, NQC):
                for i in range(4):
                    q = nc.sync if jc == 1 else nc.scalar
                    q.dma_start(xq_t[jc][i][:], xq_v[4 * jc + i])
            nc.sync.dma_start(wo_sb[:], woT.ap())
            nc.scalar.dma_start(qres_sb[:], qres.ap())
            if apply_gb:
                nc.scalar.dma_start(gam_sb[:], gam.ap())
                nc.scalar.dma_start(bet_sb[:], bet.ap())
            nc.vector.memset(onesP[:], 1.0)
            nc.scalar.dma_start(sel2[:], selc.ap())
            nc.scalar.dma_start(gm_sb[:], gmsk.ap())
            # group-mask broadcast on the (otherwise idle) gpsimd engine
            nc.gpsimd.memset(gmb8[:], 1.0)
            for g in range(2):
                nc.gpsimd.tensor_scalar_mul(
                    gmb8[:, g, :], gmb8[:, g, :], gm_sb[:, g : g + 1]
                )
            nc.vector.memset(dcp[:], 1.0)
            nc.vector.memset(rdf[:], 1.0)
            nc.vector.memset(v2[:, :, :, :, 64:65], 1.0)

            # -------- Phase A: K' qc0, Q' ch0, K' qc1-3, V kt0-3 ----------
            # ordered so the first exp (needs K' qc0 + Q' ch0) fires as
            # early as possible; V kt4-15 stream as fillers inside chunk 0
            with tc.tile_pool(name="psA", bufs=8, space="PSUM") as psA:
                def k_block(qc):
                    tiles = []
                    for dt in range(2):
                        t = psA.tile(
                            [P, QCH], F32, tag="pj", name=f"pk_{qc}_{dt}"
                        )
                        tiles.append(t)
                    for i in range(4):
                        for dt in range(2):
                            nc.tensor.matmul(
                                tiles[dt][:],
                                lhsT=wk_sb[:, i, :, P * dt : P * dt + P],
                                rhs=xkv_p[i][:, :, QCH * qc : QCH * qc + QCH],
                                perf_mode=DROW,
                                start=(i == 0),
                                stop=(i == 3),
                            )
                    qsl = slice(QCH * qc, QCH * qc + QCH)
                    for dt in range(2):
                        for hh in range(2):
                            h = 2 * dt + hh
                            nc.vector.tensor_scalar(
                                kt8[0:DK, h, qsl],
                                tiles[dt][DK * hh : DK * hh + DK, :],
                                1.0,
                                bks_sb[:, h : h + 1],
                                MUL,
                                ADD,
                            )

                def v_tile_mms(ps, kt, lo, hi):
                    # lo/hi index m-PAIRS (0..4)
                    pv = ps[:, 0:DLOC]
                    for i in range(lo, hi):
                        nc.tensor.matmul(
                            pv,
                            lhsT=xkv_p[i][:, :, P * kt : P * kt + P],
                            rhs=wv_sb[:, i, :, :],
                            perf_mode=DROW,
                            start=(i == 0),
                            stop=False,
                        )
                    if hi == 4:
                        nc.tensor.matmul(
                            pv,
                            lhsT=onesP[0:1, :],
                            rhs=bvr_sb[0:1, :],
                            start=False,
                            stop=True,
                        )

                def v_copyback(ps, kt):
                    nc.vector.tensor_copy(
                        v2[:, kt // 2, :, kt % 2, 0:64],
                        ps[:, 0:DLOC].rearrange("p (h d) -> p h d", d=DK),
                    )

                k_block(0)
                pq = [
                    psA.tile([P, QCH], F32, tag="pj", name=f"pq_{i}")
                    for i in range(2)
                ]
                for i in range(4):
                    for dt in range(2):
                        nc.tensor.matmul(
                            pq[dt][:],
                            lhsT=wq_sb[:, i, :, P * dt : P * dt + P],
                            rhs=xq_t[0][i][:],
                            perf_mode=DROW,
                            start=(i == 0),
                            stop=(i == 3),
                        )
                for dt in range(2):
                    for hh in range(2):
                        h = 2 * dt + hh
                        nc.vector.tensor_scalar(
                            qt8[0:DK, h, 0:QCH],
                            pq[dt][DK * hh : DK * hh + DK, :],
                            1.0,
                            bqs_sb[:, h : h + 1],
                            MUL,
                            ADD,
                        )
                for qc in range(1, NQC):
                    k_block(qc)
                for kt in range(NKT // 2):
                    ps = psA.tile([P, QCH], F32, tag="pj", name=f"pv_{kt}")
                    v_tile_mms(ps, kt, 0, 4)
                    v_copyback(ps, kt)
                # re-align cores after the HBM-contended prefix so the
                # per-chunk AllToAlls don't inherit the DMA skew; the
                # (all-zero) result lands in a dead-but-read corner of
                # qt8 so chunk 1's first score matmul gates on it and
                # every core actually waits for the slowest
                wsync2_in = dram.tile([NCORES, 16], F32, name="wsync2_in")
                wsync2_out = dram.tile([NCORES, 16], F32, name="wsync2_out")
                nc.gpsimd.dma_start(wsync2_in, wsrc[:])
                nc.gpsimd.collective_compute(
                    "AllToAll",
                    mybir.AluOpType.bypass,
                    replica_groups=[list(range(NCORES))],
                    ins=[wsync2_in.opt()],
                    outs=[wsync2_out.opt()],
                )



            # ------- Phase B: exp-bound attention with fillers -------
            with (
                tc.tile_pool(name="opool", bufs=1) as opool,
                tc.tile_pool(name="psB", bufs=1, space="PSUM") as psB,
            ):
                fillers = deque()
                ctf_tiles = {}
                a2a_in_tiles = {}
                x_tiles = {}
                po_cache = {}
                aux_toggle = [0]

                def aux_tile(name):
                    # two auxiliary psum banks, round-robin, so back-to-back
                    # o-proj / broadcast matmuls double-buffer
                    aux_toggle[0] ^= 1
                    tag = "aux" if aux_toggle[0] else "vq"
                    return psB.tile([P, QCH], F32, tag=tag, bufs=1, name=name)

                def po_tile(jc, nch):
                    key = (jc, nch)
                    if key not in po_cache:
                        po_cache[key] = aux_tile(f"po_{jc}_{nch}")
                    return po_cache[key]

                # ---- filler generators ----
                def v_steps(kt):
                    ps_box = {}

                    def a():
                        ps_box["t"] = aux_tile(f"pvf_{kt}")
                        v_tile_mms(ps_box["t"], kt, 0, 2)

                    def b():
                        v_tile_mms(ps_box["t"], kt, 2, 4)

                    def c():
                        v_copyback(ps_box["t"], kt)

                    return [a, b, c]

                def qproj_steps(jc, dt):
                    qsl = slice(QCH * jc, QCH * jc + QCH)
                    ps_box = {}

                    def a(lo, hi):
                        if lo == 0:
                            ps_box["t"] = aux_tile(f"pqf_{jc}_{dt}")
                        for i in range(lo, hi):
                            nc.tensor.matmul(
                                ps_box["t"][:],
                                lhsT=wq_sb[:, i, :, P * dt : P * dt + P],
                                rhs=xq_t[jc][i][:],
                                perf_mode=DROW,
                                start=(i == 0),
                                stop=(i == 3),
                            )

                    def c():
                        for hh in range(2):
                            h = 2 * dt + hh
                            nc.vector.tensor_scalar(
                                qt8[0:DK, h, qsl],
                                ps_box["t"][DK * hh : DK * hh + DK, :],
                                1.0,
                                bqs_sb[:, h : h + 1],
                                MUL,
                                ADD,
                            )

                    return [lambda: a(0, 2), lambda: a(2, 4), c]

                def norm_head_emit(ctu, cx0, cx1):
                    """Free the ctx psum banks at a head-pair boundary:
                    copy denominator rows + unnormalized context to SBUF
                    (DVE only)."""
                    nc.vector.tensor_copy(dcp[0:1, :], cx0[64:65, :])
                    nc.vector.tensor_copy(dcp[32:33, :], cx1[64:65, :])
                    nc.vector.tensor_copy(ctu[0:64, :], cx0[0:64, :])
                    nc.vector.tensor_copy(ctu[64:128, :], cx1[0:64, :])

                def norm_steps(jc, hp, ctu, tail=False):
                    """Deferred tail of the normalization, in 128-column
                    pieces so no single step head-blocks the PE queue: per
                    piece a reciprocal of the two denominator rows
                    (partitions 0/32), a row-broadcast matmul, and the
                    normalize-multiply into fp8 ct."""
                    qoff = QCH * jc
                    box = {}

                    def alloc():
                        if tail:
                            # exp stream is done; borrow a score psum bank
                            # so the held po banks stay untouched
                            box["bcp"] = psB.tile(
                                [P, 2, QCH], F32, tag="s", bufs=2,
                                name=f"bcpt_{jc}_{hp}",
                            )[:, 0, :]
                        else:
                            box["bcp"] = aux_tile(f"bcp_{jc}_{hp}")

                    def rp(q4):
                        csl = slice(P * q4, P * q4 + P)
                        nc.vector.reciprocal(rdf[0:33, csl], dcp[0:33, csl])
                        nc.vector.tensor_copy(rdb[0:33, csl], rdf[0:33, csl])

                    def bp(q4):
                        if q4 == 0:
                            alloc()
                        csl = slice(P * q4, P * q4 + P)
                        nc.tensor.matmul(
                            box["bcp"][:, csl],
                            lhsT=sel2[0:33, :],
                            rhs=rdb[0:33, csl],
                        )
                        nc.vector.tensor_mul(
                            ct_sb[:, hp, qoff + P * q4 : qoff + P * q4 + P],
                            ctu[:, csl],
                            box["bcp"][:, csl],
                        )

                    return [
                        lambda q4=q4, f=f: f(q4)
                        for q4 in range(4)
                        for f in (rp, bp)
                    ]

                def ctf_tile(jc):
                    if jc not in ctf_tiles:
                        ctf_tiles[jc] = opool.tile(
                            [P, GSZ, 2, P], FP8, tag="cf", bufs=2,
                            name=f"ctf_{jc}",
                        )
                    return ctf_tiles[jc]

                def exchange_steps(jc, hps=(0, 1), skip_stage=False):
                    """Ship chunk jc's normalized context through one 8-wide
                    AllToAll (block j = our ctx for the q-rows owned by rank
                    j's position in its group), fp8 payload.  Cross-batch
                    blocks arrive as garbage; the masked combine (0/1 group
                    mask, exact in fp8) keeps the four same-group blocks on
                    the otherwise-idle gpsimd engine.  `hps` selects which
                    head-pair halves ship (the last chunk ships hp0 inside
                    the exp stream)."""
                    nh = len(hps)
                    sfx = f"{jc}_{hps[0]}{nh}"
                    a2a_in = dram.tile([NCORES, P, nh, P], FP8, name=f"a2a_in_{sfx}")
                    a2a_out = dram.tile([NCORES, P, nh, P], FP8, name=f"a2a_out_{sfx}")
                    ctf8 = opool.tile(
                        [P, NCORES, nh, P], FP8, tag=f"c8_{nh}", bufs=2
                    )
                    ctf = ctf_tile(jc)
                    h0 = hps[0]

                    def st(lo):
                        for j in range(lo, lo + 4):
                            qo = QCH * jc + P * (j % GSZ)
                            nc.sync.dma_start(
                                a2a_in[j],
                                ct_sb[:, h0 : h0 + nh, qo : qo + P],
                            )

                    def a2a():
                        nc.gpsimd.collective_compute(
                            "AllToAll",
                            mybir.AluOpType.bypass,
                            replica_groups=[list(range(NCORES))],
                            ins=[a2a_in.opt()],
                            outs=[a2a_out.opt()],
                        )

                    def load():
                        av = a2a_out.rearrange("r p t q -> p r t q")
                        nc.sync.dma_start(ctf8[:, 0:GSZ, :, :], av[:, 0:GSZ])
                        nc.sync.dma_start(ctf8[:, GSZ:, :, :], av[:, GSZ:])

                    fsz = GSZ * nh * P

                    def gmv(g):
                        return gmb8[:, g, 0:fsz].rearrange(
                            "p (a b c) -> p a b c", a=GSZ, b=nh
                        )

                    cdst = ctf[:, :, h0 : h0 + nh, :]

                    def comb1():
                        nc.vector.tensor_tensor(
                            ctf8[:, 0:GSZ, :, :], ctf8[:, 0:GSZ, :, :],
                            gmv(0), MUL,
                        )

                    def comb2():
                        nc.vector.tensor_tensor(
                            cdst, ctf8[:, GSZ : 2 * GSZ, :, :], gmv(1), MUL
                        )
                        nc.vector.tensor_add(cdst, cdst, ctf8[:, 0:GSZ, :, :])

                    a2a_in_tiles[(jc, hps[0], nh)] = a2a_in
                    if skip_stage:
                        return [a2a, load, comb1, comb2]
                    return [lambda: st(0), lambda: st(4), a2a, load, comb1, comb2]

                def oproj_steps(jc):
                    """Full-depth o-projection for our own 128 rows of chunk
                    jc: the four same-group peers' head-pair-paired context
                    slabs contract against fp8 o-weights in DoubleRow mode."""
                    ctf = ctf_tile(jc)

                    def grp(nch, lo):
                        po = po_tile(jc, nch)
                        nsl = slice(QCH * nch, QCH * nch + QCH)
                        for r in range(lo, lo + 2):
                            nc.tensor.matmul(
                                po[:],
                                lhsT=ctf[:, r, :, :],
                                rhs=wo_sb[:, r, :, nsl],
                                perf_mode=DROW,
                                start=(r == 0),
                                stop=(r == GSZ - 1),
                            )

                    return [
                        lambda: grp(0, 0),
                        lambda: grp(0, 2),
                        lambda: grp(1, 0),
                        lambda: grp(1, 2),
                    ]

                def oproj_adds(jc):
                    x_sb = opool.tile([P, D], F32, tag="x", bufs=2)
                    x_tiles[jc] = x_sb

                    def add(nch):
                        po = po_tile(jc, nch)
                        nsl = slice(QCH * nch, QCH * nch + QCH)
                        nc.vector.tensor_add(
                            x_sb[:, nsl], po[:], qres_sb[:, jc, nsl]
                        )

                    return [lambda nch=nch: add(nch) for nch in range(2)]

                def epilogue_steps(jc):
                    """LayerNorm for chunk jc's 128 rows.  DVE-only (rsqrt
                    via reciprocal seed + Newton) so the Scalar engine never
                    reloads activation tables."""
                    x_sb = x_tiles[jc]
                    y_sb = opool.tile([P, D], F32, tag="y", bufs=2)
                    yb_sb = opool.tile([P, D], BF16, tag="yb", bufs=2)
                    stat = spool.tile([P, 2, 6], F32, tag="stat")
                    mv = spool.tile([P, 2], F32, tag="mv")
                    var = spool.tile([P, 1], F32, tag="var")
                    yy = spool.tile([P, 1], F32, tag="yy")
                    tt = spool.tile([P, 1], F32, tag="tt")
                    vh = spool.tile([P, 1], F32, tag="vh")
                    mu = mv[:, 0:1]

                    def e2():
                        # mean/variance via the BN stats unit (512-wide max)
                        nc.vector.bn_stats(stat[:, 0, :], x_sb[:, 0 : D // 2])
                        nc.vector.bn_stats(stat[:, 1, :], x_sb[:, D // 2 :])

                    def e3():
                        nc.vector.bn_aggr(mv[:], stat[:])

                    def e4():
                        nc.vector.tensor_scalar_add(var[:], mv[:, 1:2], LN_EPS)
                        nc.vector.reciprocal(yy[:], var[:])
                        nc.vector.tensor_scalar_mul(vh[:], var[:], -0.5)
                        for _ in range(3):
                            nc.vector.tensor_mul(tt[:], yy[:], yy[:])
                            nc.vector.tensor_scalar(tt[:], tt[:], vh[:], 1.5, MUL, ADD)
                            nc.vector.tensor_mul(yy[:], yy[:], tt[:])

                    def e5():
                        if apply_gb:
                            nc.vector.tensor_scalar(
                                y_sb[:], x_sb[:], mu, yy[:], SUB, MUL
                            )
                            nc.vector.tensor_mul(y_sb[:], y_sb[:], gam_sb[:])
                            nc.vector.tensor_add(yb_sb[:], y_sb[:], bet_sb[:])
                            nc.sync.dma_start(out.ap()[jc], yb_sb[:])
                        else:
                            for h in range(2):
                                csl = slice(D // 2 * h, D // 2 * (h + 1))
                                nc.vector.tensor_scalar(
                                    yb_sb[:, csl], x_sb[:, csl], mu, yy[:], SUB, MUL
                                )
                                nc.sync.dma_start(out.ap()[jc][:, csl], yb_sb[:, csl])

                    return [e2, e3, e4, e5]

                # V k-tiles 8-15 and Q' chunks 1-3 fill chunk 0's attention
                for kt in range(NKT // 2, NKT):
                    fillers.extend(v_steps(kt))
                for dt in range(2):
                    fillers.extend(qproj_steps(1, dt))

                # ---- the exp-bound attention loop ----
                def post_steps(k):
                    return oproj_steps(k) + oproj_adds(k) + epilogue_steps(k)

                pend = {}
                # cross-stream carry: the last ctx pair + norm_head copies
                # of stream N are emitted after stream N+1's first scores
                # (PE never head-blocks on exp(kt15) at a boundary); the
                # new stream's cx psum tiles are allocated AFTER the carry
                # flush so the pool orders bank reuse behind the old reads
                xcarry = [None]
                for jc in range(NQC):
                    if jc >= 1:
                        # norm steps must stay contiguous: the bcp psum
                        # bank allocated at the first bp piece shares the
                        # two aux banks with post_steps' held po tiles
                        nst = norm_steps(jc - 1, 1, pend[(jc - 1, 1)])
                        fillers.extend(nst)
                        if jc == NQC - 1:
                            # chunk 1's post-exchange work: 1.5 chunks after
                            # its AllToAll so collective jitter never
                            # head-blocks the PE queue
                            fillers.extend(post_steps(1))
                        fillers.extend(exchange_steps(jc - 1))
                        if jc + 1 < NQC:
                            # Q' for chunk jc+1: chunk 0 is PE-bound, later
                            # chunks have spare PE slack
                            for dt in range(2):
                                fillers.extend(qproj_steps(jc + 1, dt))
                    qsl = slice(QCH * jc, QCH * jc + QCH)
                    for hp in range(2):
                        if hp == 1:
                            nst = norm_steps(jc, 0, pend[(jc, 0)])
                            fillers.extend(nst)
                            if jc == NQC - 2:
                                fillers.extend(post_steps(0))
                            if jc == NQC - 1:
                                # ship hp0 of the last chunk as early as
                                # possible (its AllToAll must complete
                                # before the tail's o-proj), but keep the
                                # load/combine late so the DVE queue never
                                # stalls behind the collective
                                ex0 = exchange_steps(jc, hps=(0,))
                                fillers.extend(ex0[:3])
                                fillers.extend(post_steps(jc - 1))
                                fillers.extend(ex0[3:])
                        cxb = {}
                        h0, h1 = 2 * hp, 2 * hp + 1
                        p2 = None
                        pend_ctx = None

                        def mk_ctx(pr, p2c, st, sp, cxb=cxb, h0=h0, h1=h1):
                            def go():
                                for i, cx, h in (
                                    (0, cxb["c0"], h0),
                                    (1, cxb["c1"], h1),
                                ):
                                    nc.tensor.matmul(
                                        cx[0:65, :],
                                        lhsT=v2[:, pr, h, :, 0:65],
                                        rhs=p2c[:, i, :, :],
                                        perf_mode=DROW,
                                        start=st,
                                        stop=sp,
                                    )

                            return go

                        for kt in range(NKT):
                            ksl = slice(P * kt, P * kt + P)
                            s = psB.tile([P, 2, QCH], F32, tag="s", bufs=2)
                            for i, h in ((0, h0), (1, h1)):
                                nc.tensor.matmul(
                                    s[:, i, :],
                                    lhsT=kt8[:, h, ksl]
                                    .unsqueeze(1)
                                    .broadcast_to([P, 2, P]),
                                    rhs=qt8[:, h, qsl]
                                    .unsqueeze(1)
                                    .broadcast_to([P, 2, QCH]),
                                    perf_mode=DROW,
                                )
                            # the ctx pair for the PREVIOUS k-tile pair is
                            # emitted after this k-tile's scores: its exp
                            # dependency resolves while the scores stream,
                            # so the in-order PE queue never head-blocks
                            if kt == 0:
                                if xcarry[0] is not None:
                                    xcarry[0]()
                                    xcarry[0] = None
                                cxb["c0"] = psB.tile(
                                    [P, QCH], F32, tag="ctx0", bufs=1,
                                    name=f"cx0_{jc}_{hp}",
                                )
                                cxb["c1"] = psB.tile(
                                    [P, QCH], F32, tag="ctx1", bufs=1,
                                    name=f"cx1_{jc}_{hp}",
                                )
                            if pend_ctx is not None:
                                pend_ctx()
                                pend_ctx = None
                            if kt % 2 == 0:
                                p2 = spool.tile(
                                    [P, 2, 2, QCH], FP8E5, tag="p", bufs=6
                                )
                            # scale: 1/sqrt(dk)=0.125 times 0.5 for the
                            # stride-0 double-count of the score contraction.
                            # e5m2 output: covers exp of scores up to 10.9
                            # with no max-subtraction needed.
                            nc.scalar.activation(
                                p2[:, :, kt % 2, :], s[:], Exp, scale=0.0625
                            )
                            if kt % 2 == 1:
                                pend_ctx = mk_ctx(
                                    kt // 2, p2, kt == 1, kt == NKT - 1
                                )
                            # keep the first/last k-tiles filler-free so the
                            # exp stream never competes at boundaries
                            if 2 <= kt <= 14:
                                n_pop = 2 if len(fillers) > 4 else 1
                                for _ in range(n_pop):
                                    if fillers:
                                        fillers.popleft()()
                        ctu = spool.tile(
                            [P, QCH], BF16, tag="ctu", bufs=3,
                            name=f"ctu_{jc}_{hp}",
                        )
                        pend[(jc, hp)] = ctu

                        def mk_carry(pc=pend_ctx, ct=ctu,
                                     c0=cxb["c0"], c1=cxb["c1"]):
                            def go():
                                pc()
                                norm_head_emit(ct, c0, c1)

                            return go

                        xcarry[0] = mk_carry()
                        pend_ctx = None

                # ---- tail ----
                if xcarry[0] is not None:
                    xcarry[0]()
                    xcarry[0] = None
                while fillers:
                    fillers.popleft()()
                jl = NQC - 1
                norm_tail = norm_steps(jl, 1, pend[(jl, 1)], tail=True)
                # per 128-column piece: recip -> broadcast+normalize -> ship
                # (alternating DMA queues so the stage DMAs overlap)
                ex = exchange_steps(jl, hps=(1,), skip_stage=True)
                a2a_in_t = a2a_in_tiles[(jl, 1, 1)]
                qoff = QCH * jl
                for q4 in range(GSZ):
                    norm_tail[2 * q4]()
                    norm_tail[2 * q4 + 1]()
                    qq = nc.sync if q4 % 2 == 0 else nc.scalar
                    for j in (q4, q4 + GSZ):
                        qq.dma_start(
                            a2a_in_t[j],
                            ct_sb[:, 1:2, qoff + P * q4 : qoff + P * q4 + P],
                        )
                for step in (
                    ex + oproj_steps(jl) + oproj_adds(jl) + epilogue_steps(jl)
                ):
                    step()

    _split_waits(nc)
    return nc


def _prep_inputs(query, key_value, W_qkv, b_qkv, W_o, b_o, ln_gamma, ln_beta,
                 apply_gb):
    bf16 = ml_dtypes.bfloat16
    f32 = np.float32
    query = np.asarray(query, f32)
    key_value = np.asarray(key_value, f32)
    W_qkv = np.asarray(W_qkv, f32)
    b_qkv = np.asarray(b_qkv, f32)
    W_o = np.asarray(W_o, f32)
    b_o = np.asarray(b_o, f32)
    ln_gamma = np.asarray(ln_gamma, f32)
    ln_beta = np.asarray(ln_beta, f32)

    fp8 = ml_dtypes.float8_e4m3
    Wq, Wk, Wv = W_qkv[:D], W_qkv[D : 2 * D], W_qkv[2 * D :]
    bq, bk, bv = b_qkv[:D], b_qkv[D : 2 * D], b_qkv[2 * D :]

    gam = np.ascontiguousarray(np.broadcast_to(ln_gamma, (P, D))).astype(f32)
    bet = np.ascontiguousarray(np.broadcast_to(ln_beta, (P, D))).astype(f32)
    sel_const = np.zeros((DK, P), f32)
    sel_const[0, 0:DK] = 1.0
    sel_const[32, DK:P] = 1.0
    sel_const = sel_const.astype(bf16)

    def pack_w(W):
        # [1024, DLOC] -> [P, 4, 2, DLOC] fp8, m = 128*(2i+j)+p
        return np.ascontiguousarray(
            W.T.reshape(4, 2, P, DLOC).transpose(2, 0, 1, 3)
        ).astype(fp8)

    # o-proj weights per group-rank block r, head-pair t on the slot dim
    wo2 = np.ascontiguousarray(
        W_o.T.reshape(GSZ, 2, P, D).transpose(2, 0, 1, 3)
    ).astype(fp8)

    xqT = [
        np.ascontiguousarray(
            query[b].T.reshape(4, 2, P, NQC, QCH)
            .transpose(3, 0, 2, 1, 4)
            .reshape(NQC * 4, P, 2, QCH)
        ).astype(fp8)
        for b in range(B)
    ]
    xkvT = [
        np.ascontiguousarray(
            key_value[b].T.reshape(4, 2, P, SKV).transpose(0, 2, 1, 3)
        ).astype(fp8)
        for b in range(B)
    ]

    in_maps = []
    for c in range(NCORES):
        b = c // GSZ
        hb = c % GSZ
        jb = c % GSZ
        sl = slice(DLOC * hb, DLOC * hb + DLOC)
        gm = np.zeros((P, 2), f32)
        gm[:, b] = 1.0
        # this core owns rows 512*jc + 128*jb .. +128 of each chunk jc
        res_rows = np.stack(
            [
                query[b, QCH * jc + P * jb : QCH * jc + P * jb + P]
                + b_o[None, :]
                for jc in range(NQC)
            ]
        ).transpose(1, 0, 2)
        im = {
            "xqT": xqT[b],
            "xkvT": xkvT[b],
            "wqT": pack_w(Wq[sl]),
            "wkT": pack_w(Wk[sl]),
            "wvT": pack_w(Wv[sl]),
            "bqs": np.ascontiguousarray(bq[sl].reshape(HLOC, DK).T).astype(f32),
            "bks": np.ascontiguousarray(bk[sl].reshape(HLOC, DK).T).astype(f32),
            "bvr": bv[sl][None, :].astype(bf16),
            "woT": wo2,
            "selc": sel_const,
            "gmsk": gm,
            "qres": res_rows.astype(bf16),
        }
        if apply_gb:
            im["gam"] = gam
            im["bet"] = bet
        in_maps.append(im)
    return in_maps


def kernel(query, key_value, W_qkv, b_qkv, W_o, b_o, ln_gamma, ln_beta):
    global LAST_RESULT
    apply_gb = not (
        np.all(np.asarray(ln_gamma) == 1.0) and np.all(np.asarray(ln_beta) == 0.0)
    )
    key = ("nc", apply_gb)
    if key not in _CACHE:
        _CACHE[key] = _build(apply_gb)
    nc = _CACHE[key]
    in_maps = _prep_inputs(
        query, key_value, W_qkv, b_qkv, W_o, b_o, ln_gamma, ln_beta, apply_gb
    )
    res = run_bass_kernel_spmd(nc, in_maps, core_ids=list(range(NCORES)))
    LAST_RESULT = res
    full = np.empty((B, SQ, D), np.float32)
    for c in range(NCORES):
        b = c // GSZ
        jb = c % GSZ
        o = np.asarray(res.results[c]["out"], np.float32)  # [NQC, P, D]
        for jc in range(NQC):
            r0 = QCH * jc + P * jb
            full[b, r0 : r0 + P] = o[jc]
    return full

